# revision 30
# baseline (speedup 1.0000x reference)
"""Trainium2 Bass kernel for the DGL-JTNN tree decoder (nn_DGLJTNNDecoder).

Strategy: pure data-parallel over the 512 trees, 64 trees per NeuronCore.

v3 redesign (vs. v2):
  - DFS lag structure exploited: pred-edge lags are always odd (1,3,5,...)
    and node-inbox lags always even (2,4,...).  Masked SBUF adds now cover
    pred lag-1/lag-3 and node lag-2/lag-4 (the old lag-2 pred / lag-1 node
    masks never fired); the DRAM step-log gather only covers lag>=5, so it
    can be issued 4 iterations early (PF=4) and round-robins over 2 SWDGE
    queues -> fully hidden (v2 stalled ~5us/step on a single-queue gather).
  - Per-step streams (Er/Ez/Eh embedding rows, masks, A-routing chunks) are
    packed into one DRAM stream loaded 4 steps per DMA: ~32 descriptors per
    step instead of ~256 (the HWDGE queues are descriptor-rate-bound).
  - Er/Ez/Eh ship TREE-major and are preloaded into PSUM by identity
    matmuls at the head of each accumulation group (PE-ordered, race-free),
    removing three vector adds per step from the critical path.
  - q-head: tree_vec projection preloaded by identity matmul (was a
    broadcast vector add per 256-col block); the softmax max-reduce is
    dropped (|logits| < 1, exp is safe); argmax match is computed as
    "count of logits > target logit" via scalar_tensor_tensor accum.
Losses/accuracies reduce to 8 partial sums per core, combined on the host.
"""

import os
import numpy as np

import concourse.bass as bass
import concourse.bacc as bacc
import concourse.mybir as mybir
import concourse.tile as tile
from concourse.library_config import mlp as _mlp_lib
from concourse.bass_utils import run_bass_kernel_spmd

f16 = mybir.dt.float16
f32 = mybir.dt.float32
i16 = mybir.dt.int16
AF = mybir.ActivationFunctionType
ALU = mybir.AluOpType

# problem constants (hardcoded per contract)
B, N, H, L, V = 512, 20, 450, 56, 780
T = 2 * (N - 1)            # 38 steps
NC = 8                     # cores
C = B // NC                # 64 trees/core
Hp = 512                   # padded hidden
NBLK = 40                  # head col blocks (39 real + 1 pad) -> 2560 cols
NCOL = NBLK * C            # 2560
RC = NCOL // 128           # 20 row chunks
P, Dn = 3, 4
LOG_ROWS = T * 128         # state log rows (step t -> rows t*128 : +128)
SG = 4                     # steps per stream-group DMA
PF = 4                     # gather prefetch depth (gather t reads steps <= t-5)
NSWQ = 2                   # SWDGE queues for gathers (round-robin)

DBG_T = int(os.environ.get("KDBG_T", T))
DBG_HEADS = os.environ.get("KDBG_HEADS", "1") == "1"
DBG_Q = os.environ.get("KDBG_Q", "1") == "1"
DBG_FIN = os.environ.get("KDBG_FIN", "1") == "1"
GPELEM = os.environ.get("KDBG_GPELEM", "0") == "1"
GFULL = 0                  # extra gather steps forced to full 128-row counts


def _wrap_idx(idx):
    """[n*16] flat gather order -> [16, n] wrapped, replicated to 128 rows."""
    idx = np.asarray(idx, np.int16)
    n = idx.shape[0] // 16
    return np.tile(idx.reshape(n, 16).T, (8, 1))    # [128, n]


def _host_prep(inputs):
    inp = {k: np.asarray(v) for k, v in inputs.items()}
    (tree_vec, emb, Wz, bz, Wh, bh, Wr, Ur, br, Ww, bw, Uw, bu, Wo, bo,
     Us, bs) = (inp[k] for k in
                ['tree_vec', 'emb', 'Wz', 'bz', 'Wh', 'bh', 'Wr', 'Ur', 'br',
                 'Ww', 'bw', 'Uw', 'bu', 'Wo', 'bo', 'Us', 'bs'])
    wid, root_ids = inp['wid'], inp['root_ids']
    edge_src, edge_dst = inp['edge_src'], inp['edge_dst']
    edge_pred, node_in = inp['edge_pred'], inp['node_in']
    step_eid, step_v = inp['step_eid'], inp['step_v']
    q_rows, q_tgt, p_tgt = inp['q_rows'], inp['q_tgt'], inp['p_tgt']
    n_edges = edge_src.shape[0]

    def padHp(M, axis):
        pads = [(0, 0)] * M.ndim
        pads[axis] = (0, Hp - M.shape[axis])
        return np.pad(M, pads)

    def h16(x):
        return np.ascontiguousarray(x.astype(np.float16))

    # ---- folded tables (fp32 math) ----
    Ez = padHp(emb @ Wz[:H] + bz, 1)
    Eh = padHp(emb @ Wh[:H] + bh, 1)
    Er = padHp(emb @ Wr + br, 1)
    Eu = padHp(emb @ Uw[:H] + bu, 1)
    Eu[:, 511] = 1.0                      # bias-injection channel for p-head

    def packW(Wm):                         # [512,512] -> [128, 4*512]
        return h16(Wm.reshape(4, 128, Hp).transpose(1, 0, 2).reshape(128, 4 * Hp))

    wz_h = packW(padHp(padHp(Wz[H:], 0), 1))
    wh_h = packW(padHp(padHp(Wh[H:], 0), 1))
    wu_h = packW(padHp(padHp(Ur, 0), 1))
    wuwh_h = packW(padHp(padHp(Uw[H:2 * H], 0), 1))
    wwwh_h = packW(padHp(padHp(Ww[:H], 0), 1))
    Wop = padHp(Wo, 0)
    Wop[511, :] = bo                      # bias row (qrelu[511]==1)
    wo_h = h16(Wop.reshape(4, 128, V).transpose(1, 0, 2).reshape(128, 4 * V))
    Usp = padHp(Us, 0)
    Usp[511, 0] = bs[0]                   # bias row (prelu[511]==1)
    us_h = h16(Usp.reshape(4, 128).T)     # [128, 4]

    # tree_vec with ones column (bias channel), host-side projections
    tvpad = np.zeros((B, 64), np.float32)
    tvpad[:, :L] = tree_vec
    tvpad[:, L] = 1.0
    WuL = np.zeros((64, Hp), np.float32)
    WuL[:L] = padHp(Uw[2 * H:], 1)
    WwL = np.zeros((64, Hp), np.float32)
    WwL[:L] = padHp(Ww[H:], 1)
    WwL[L, :H] = bw                       # ones channel -> +bw
    WwL[L, 511] = 1.0                     # makes qrelu[511]==1

    estep = np.full(n_edges, -1, np.int64)
    for t in range(T):
        for b in range(B):
            estep[step_eid[t, b]] = t

    # ---- per-core graph metadata ----
    # masks layout per step: P1m 0:64 | P1rm 64:128 | P3 128:256 (m|rm) |
    #                        N2 256:320 | N4 320:384
    raw = []
    for core in range(NC):
        trees = np.arange(core * C, (core + 1) * C)
        masks = np.zeros((T, 384), np.float16)
        grows = [dict() for _ in range(T)]            # log row -> set(out cols)
        for t in range(T):
            for j, b in enumerate(trees):
                e = step_eid[t, b]
                v = step_v[t, b]
                for p in range(P):
                    pe = edge_pred[e, p]
                    if pe >= n_edges:
                        continue
                    tp = estep[pe]
                    if tp > t:
                        continue
                    lag = t - tp
                    if lag == 1:
                        masks[t, j] = 1.0; masks[t, 64 + j] = 1.0
                    elif lag == 3:
                        masks[t, 128 + j] = 1.0; masks[t, 192 + j] = 1.0
                    else:
                        grows[t].setdefault(tp * 128 + j, set()).add(j)
                        grows[t].setdefault(tp * 128 + 64 + j, set()).add(64 + j)
                for p in range(Dn):
                    ie = node_in[v, p]
                    if ie >= n_edges or ie == e:
                        continue
                    ti = estep[ie]
                    if ti > t:
                        continue
                    lag = t - ti
                    if lag == 2:
                        masks[t, 256 + j] = 1.0
                    elif lag == 4:
                        masks[t, 320 + j] = 1.0
                    else:
                        grows[t].setdefault(ti * 128 + j, set()).add(128 + j)

        # head Eu stream column word ids
        widrow = np.zeros(NCOL, np.int64)
        widrow[:C] = wid[root_ids[trees]]
        for t in range(T):
            widrow[(t + 1) * 64:(t + 2) * 64] = wid[step_v[t, trees]]

        ws = wid[edge_src[step_eid[:, trees]]]        # [T, C]
        wd = wid[edge_dst[step_eid[:, trees]]]

        # q/p loss tables, row-major [128, 20]
        qmask = np.zeros((128, RC), np.float32)
        qtg = np.zeros((128, RC), np.float32)
        ptgt = np.zeros((128, RC), np.float32)
        pmask = np.zeros((128, RC), np.float32)
        for i in range(q_rows.shape[0]):
            g = int(q_rows[i])
            k, b = g // B, g % B
            if core * C <= b < (core + 1) * C:
                l = k * C + (b - core * C)
                qmask[l % 128, l // 128] = 1.0
                qtg[l % 128, l // 128] = float(q_tgt[i])
        for l in range(39 * C):
            k, j = l // C, l % C
            g = k * B + core * C + j
            ptgt[l % 128, l // 128] = float(p_tgt[g])
            pmask[l % 128, l // 128] = 1.0

        raw.append(dict(trees=trees, masks=masks, grows=grows, widrow=widrow,
                        ws=ws, wd=wd, qtg=qtg, qmask=qmask, ptgt=ptgt,
                        pmask=pmask))

    # shared per-step chunk counts (program structure; max over cores)
    nch = tuple(max((len(r['grows'][t]) + 127) // 128 for r in raw)
                for t in range(T))
    AW = sum(nch)
    aoffs = np.concatenate([[0], np.cumsum(nch)]).astype(np.int64)
    # shared per-step gather counts (compile-time constants; max over cores,
    # rounded up to 16).  Per-core idx arrays pad real rows up to the shared
    # count with valid row 0, then -1 (ucode: count == #non-negative idxs).
    gstep0 = next((t for t in range(T) if nch[t]), T)
    gcnts = []
    for t in range(T):
        if nch[t] == 0:
            gcnts.append(0)
        elif t < gstep0 + GFULL:
            gcnts.append(nch[t] * 128)
        else:
            mx = max(len(r['grows'][t]) for r in raw)
            gcnts.append(min(max(16, (mx + 15) // 16 * 16), nch[t] * 128))
    gcnts = tuple(gcnts)
    NCHMAX = max(nch) if nch else 0
    # stream E-part (64 tree partitions): er_tm 512 | ez_tm 512 | eh_tm 512
    # stream M-part (128 partitions): masks 384 | at NCHMAX*192
    WSM = 384 + 192 * NCHMAX
    NG = (T + SG - 1) // SG

    # ---- per-core input tensors ----
    cores = []
    for core in range(NC):
        r = raw[core]
        gidx_w = np.zeros((128, max(AW, 1) * 8), np.int16)
        strmE = np.zeros((64, NG * SG * 1536), np.float16)
        strmM = np.zeros((128, NG * SG * WSM), np.float16)
        for t in range(T):
            n = nch[t]
            be = t * 1536
            strmE[:, be:be + 512] = Er[r['wd'][t]]
            strmE[:, be + 512:be + 1024] = Ez[r['ws'][t]]
            strmE[:, be + 1024:be + 1536] = Eh[r['ws'][t]]
            bm = t * WSM
            strmM[:, bm:bm + 384] = np.broadcast_to(r['masks'][t], (128, 384))
            if n == 0:
                continue
            rows = sorted(r['grows'][t].keys())
            cnt16 = gcnts[t]
            # ucode contract: count reg == #non-negative idxs.  Pad the
            # real rows up to the shared cnt16 with valid row 0, then -1.
            idx = np.full(n * 128, -1, np.int64)
            idx[:cnt16] = 0
            idx[:len(rows)] = rows
            gidx_w[:, aoffs[t] * 8:(aoffs[t] + n) * 8] = _wrap_idx(idx)
            A = np.zeros((n * 128, 192), np.float16)
            for i, row in enumerate(rows):
                for col in r['grows'][t][row]:
                    A[i, col] = 1.0
            # gathered out[p, c, :] = log[idx[c*128+p]] -> A chunk c row p
            for c in range(n):
                strmM[:, bm + 384 + c * 192:bm + 384 + (c + 1) * 192] = \
                    A[c * 128:(c + 1) * 128]

        # tree_vec projections
        tv = tvpad[r['trees']]                         # [C, 64]
        tun = (tv @ WuL).reshape(C, 4, 128).transpose(2, 1, 0)   # feature-major
        twn_tm = np.zeros((128, Hp), np.float32)
        twn_tm[:C] = tv @ WwL                          # tree-major [128, 512]

        # idt4: 64-identity tiled 4x horizontally [128, 256]
        idt4 = np.zeros((128, 256), np.float16)
        for j in range(C):
            for k4 in range(4):
                idt4[j, k4 * 64 + j] = 1.0

        # eus stream: per cc [128, 4, 512] = Eu[widrow]+tun folded
        eus = np.zeros((128, 5 * 2048), np.float16)
        for cc in range(5):
            eu = Eu[r['widrow'][cc * 512:(cc + 1) * 512]]
            eu = eu.reshape(512, 4, 128).transpose(2, 1, 0)   # [128, 4, 512]
            eu = eu + np.tile(tun, (1, 1, 8))
            eus[:, cc * 2048:(cc + 1) * 2048] = eu.reshape(128, 2048)

        cores.append(dict(
            wz=wz_h, wh=wh_h, wu=wu_h, wuwh=wuwh_h, wwwh=wwwh_h,
            wo=wo_h, us=us_h,
            strmE=strmE, strmM=strmM, gidx=gidx_w,
            eus=eus,
            twn=np.ascontiguousarray(twn_tm.astype(np.float16)),
            idt4=idt4,
            qtg=r['qtg'], qmask=r['qmask'], ptgt=r['ptgt'], pmask=r['pmask'],
            iota=np.broadcast_to(np.arange(V, dtype=np.float32), (128, V)).copy(),
        ))
    return cores, nch, gcnts


def _build_program(nch, gcnts):
    AW = sum(nch)
    NCHMAX = max(nch) if nch else 0
    WSM = 384 + 192 * NCHMAX
    NG = (T + SG - 1) // SG
    aoffs = np.concatenate([[0], np.cumsum(np.asarray(nch, np.int64))])
    nc = bacc.Bacc("TRN2", debug=False, num_swdge_queues=NSWQ)
    GPE = nc.gpsimd if GPELEM else nc.vector
    i32 = mybir.dt.int32

    D = {}
    def di(name, shape, dt):
        D[name] = nc.dram_tensor(name, shape, dt, kind="ExternalInput")
        return D[name]

    for w in ["wz", "wh", "wu", "wuwh", "wwwh"]:
        di(w, [128, 4 * Hp], f16)
    di("wo", [128, 4 * V], f16)
    di("us", [128, 4], f16)
    di("strmE", [64, NG * SG * 1536], f16)
    di("strmM", [128, NG * SG * WSM], f16)
    di("gidx", [128, max(AW, 1) * 8], i16)
    di("eus", [128, 5 * 2048], f16)
    di("twn", [128, Hp], f16)
    di("idt4", [128, 256], f16)
    for x in ["qtg", "qmask", "ptgt", "pmask"]:
        di(x, [128, RC], f32)
    di("iota", [128, V], f32)
    out_d = nc.dram_tensor("out", [1, 8], f32, kind="ExternalOutput")

    mlog = nc.dram_tensor("mlog", [LOG_ROWS, Hp], f16, kind="Internal")
    pl_dram = nc.dram_tensor("pl_scratch", [1, NCOL], f32, kind="Internal")

    with tile.TileContext(nc) as tc:
        with tc.tile_pool(name="const", bufs=1) as cp:
            nc.gpsimd.load_library(_mlp_lib)
            # ---- early constants (scan) on sync queue ----
            def ld(name, shape, dt, eng=None):
                t_ = cp.tile(shape, dt, tag=name)
                eng = eng or nc.sync
                eng.dma_start(out=t_[:], in_=D[name][:].rearrange(
                    "p (a b) -> p a b", a=shape[1]) if len(shape) == 3 else D[name][:])
                return t_
            wz = ld("wz", [128, 4, Hp], f16)
            wh = ld("wh", [128, 4, Hp], f16)
            wu = ld("wu", [128, 4, Hp], f16)
            gidx = ld("gidx", [128, max(AW, 1), 8], i16) if AW else None
            rg = {c: nc.gpsimd.to_reg(c) for c in sorted(set(gcnts)) if c}

            ident = cp.tile([128, 128], f16)
            from concourse.masks import make_identity
            make_identity(nc, ident[:])

            # h slab (feature-major), zeroed (roots + pad cols)
            hslab = cp.tile([128, 4, NCOL], f16)
            nc.vector.memset(hslab[:], 0.0)

            # persistent E-stream tiles (manual 3-rotation): partitions
            # 64:128 are never DMA'd -- zero them once so the identity-rhs
            # matmuls contract finite*0 (stale NaN*0 would poison PSUM)
            esl = []
            for i in range(3):
                e_ = cp.tile([128, SG, 1536], f16, tag=f"esl{i}")
                nc.vector.memset(e_[64:128, :, :], 0.0)
                esl.append(e_)

            # ---------------- scan ----------------
            # Per loop iteration t we emit:
            #   stream(group):  4-step stream DMA at group boundaries
            #   tail(t-1):     d-chain -> Mt, r-gate (er-preload mm), rm
            #   prep(t+1):     A-matmul -> S_ps, S/hp extraction
            #   lagL(t+1):     P3/N2/N4 masked adds (need M(t-1) at most)
            #   head(t):       P1 adds + z/h matmul groups (ez/eh preload)
            #   wb(t-1):       hslab, transpose, log write (off-chain)
            #   prefetch(t+PF): gather (round-robin SWDGE queues)
            with tc.tile_pool(name="strm", bufs=3) as sp, \
                 tc.tile_pool(name="g", bufs=6) as gp, \
                 tc.tile_pool(name="wk", bufs=4) as wkp, \
                 tc.tile_pool(name="mrm", bufs=6) as mrmp, \
                 tc.tile_pool(name="sps", bufs=1, space="PSUM") as sps, \
                 tc.tile_pool(name="scps", bufs=2, space="PSUM") as scps:
                M_hist = []

                def load_group(g):
                    if g * SG >= DBG_T or g >= NG:
                        return None
                    e_ = esl[g % 3]
                    nc.sync.dma_start(
                        out=e_[0:64, :, :],
                        in_=D["strmE"][:, g * SG * 1536:(g + 1) * SG * 1536]
                        .rearrange("p (s w) -> p s w", s=SG))
                    m_ = sp.tile([128, SG, WSM], f16, tag="strm")
                    nc.sync.dma_start(
                        out=m_[:],
                        in_=D["strmM"][:, g * SG * WSM:(g + 1) * SG * WSM]
                        .rearrange("p (s w) -> p s w", s=SG))
                    return m_

                sgrp = {}
                for g0 in range(2):
                    sgrp[g0] = load_group(g0)

                def stE(t, a, b):
                    return esl[(t // SG) % 3][:, t % SG, a:b]

                def stM(t, a, b):
                    return sgrp[t // SG][:, t % SG, a:b]

                def prefetch(tf):
                    """gather for step tf (reads log steps <= tf-5)."""
                    if tf >= DBG_T or not nch[tf]:
                        return None
                    ncht = nch[tf]
                    g = gp.tile([128, ncht, Hp], f16, tag="g")
                    ao = int(aoffs[tf])
                    nc.gpsimd.dma_gather(
                        g[:], mlog[0:max(tf - 4, 1) * 128, :],
                        gidx[:, ao:ao + ncht, :].rearrange(
                            "p a b -> p (a b)"),
                        ncht * 128, rg[gcnts[tf]], Hp,
                        transpose=False, queue_num=tf % NSWQ)
                    return g

                def prep(ts_, g):
                    """A-matmul + merged S|hp extraction for step ts_.
                    S layout: [:, :, 0:64] s, [64:128] arm, [128:192] hp."""
                    if ts_ >= DBG_T:
                        return None
                    ncht = nch[ts_]
                    S = wkp.tile([128, 4, 192], f16, tag="S")
                    if ncht:
                        at = stM(ts_, 384, 384 + ncht * 192).rearrange(
                            "p (c n) -> p c n", c=ncht)
                        S_ps = sps.tile([128, 4, 256], f32, space="PSUM",
                                        tag="sps")
                        # contract only the gathered rows: chunk c covers
                        # gathered rows [c*128, c*128+kc) -- rows past the
                        # per-step count are never written (stale SBUF)
                        cnt = gcnts[ts_]
                        for m in range(4):
                            for c in range(ncht):
                                kc = min(cnt - c * 128, 128)
                                if kc <= 0:
                                    continue
                                nc.tensor.matmul(S_ps[:, m, 0:192],
                                                 lhsT=g[0:kc, c,
                                                        bass.ts(m, 128)],
                                                 rhs=at[0:kc, c, :],
                                                 start=(c == 0),
                                                 stop=(c == ncht - 1 or
                                                       cnt - c * 128 <= 128))
                        nc.scalar.activation(S[:], S_ps[:, :, 0:192], AF.Copy)
                    else:
                        nc.vector.memset(S[:], 0.0)
                    return {'S': S}

                def lag_late(ts_, cur):
                    """P3/N2/N4 masked adds for step ts_ (need M(ts_-2))."""
                    if cur is None:
                        return
                    S = cur['S']
                    if ts_ >= 3:
                        Mp = M_hist[ts_ - 3]
                        lg3 = wkp.tile([128, 4, 128], f16, tag="lg3")
                        nc.vector.tensor_mul(lg3[:], Mp[:],
                                             stM(ts_, 128, 256)
                                             .rearrange("p (o n) -> p o n", o=1)
                                             .to_broadcast([128, 4, 128]))
                        nc.vector.tensor_add(S[:, :, 0:128], S[:, :, 0:128],
                                             lg3[:])
                    if ts_ >= 2:
                        Mp = M_hist[ts_ - 2]
                        ln2 = wkp.tile([128, 4, 64], f16, tag="ln2")
                        GPE.tensor_mul(ln2[:], Mp[:, :, 0:64],
                                       stM(ts_, 256, 320)
                                       .rearrange("p (o n) -> p o n", o=1)
                                       .to_broadcast([128, 4, 64]))
                        GPE.tensor_add(S[:, :, 128:192], S[:, :, 128:192],
                                       ln2[:])
                    if ts_ >= 4:
                        Mp = M_hist[ts_ - 4]
                        ln4 = wkp.tile([128, 4, 64], f16, tag="ln4")
                        GPE.tensor_mul(ln4[:], Mp[:, :, 0:64],
                                       stM(ts_, 320, 384)
                                       .rearrange("p (o n) -> p o n", o=1)
                                       .to_broadcast([128, 4, 64]))
                        GPE.tensor_add(S[:, :, 128:192], S[:, :, 128:192],
                                       ln4[:])

                pend = {}
                for tf in range(min(PF, DBG_T)):
                    pend[tf] = prefetch(tf)
                st = {0: prep(0, pend.get(0))}
                lag_late(0, st[0])
                carry = None
                for t in range(DBG_T + 1):
                    if t < DBG_T:
                        cur = st.pop(t)
                    # stream prefetch (t%SG==1: the evicted group's last
                    # reader was tail(t-1) of the previous iteration)
                    if t < DBG_T and t % SG == 1:
                        g2 = t // SG + 2
                        if g2 not in sgrp:
                            sgrp[g2] = load_group(g2)

                    # ---- tail(t-1): d-chain -> M, r-gate, rm ----
                    if carry is not None:
                        tl = t - 1
                        pS, z_t, th_t = carry['S'], carry['z'], carry['th']
                        Mt = mrmp.tile([128, 4, 128], f16, tag="M")
                        d_t = wkp.tile([128, 4, 64], f16, tag="d")
                        nc.vector.tensor_sub(d_t[:], th_t[:], pS[:, :, 0:64])
                        nc.vector.tensor_mul(d_t[:], z_t[:], d_t[:])
                        nc.vector.tensor_add(Mt[:, :, 0:64], d_t[:],
                                             pS[:, :, 0:64])
                        r_ps = scps.tile([128, 4, 64], f32, space="PSUM",
                                         tag="r")
                        er = stE(tl, 0, 512).rearrange(
                            "p (a b) -> p a b", a=4)
                        for m in range(4):
                            nc.tensor.matmul(r_ps[:, m, :],
                                             lhsT=er[:, m, :],
                                             rhs=ident[:, 0:64],
                                             start=True, stop=False)
                            for k in range(4):
                                nc.tensor.matmul(r_ps[:, m, :],
                                                 lhsT=wu[:, k, bass.ts(m, 128)],
                                                 rhs=Mt[:, k, 0:64],
                                                 start=False, stop=(k == 3))
                        r_t = wkp.tile([128, 4, 64], f16, tag="rt")
                        nc.scalar.activation(r_t[:], r_ps[:], AF.Sigmoid)
                        nc.vector.tensor_mul(Mt[:, :, 64:128], r_t[:],
                                             Mt[:, :, 0:64])
                        M_hist.append(Mt)

                    # ---- head(t): P1 add + z/h groups + activations ----
                    if t < DBG_T:
                        S = cur['S']
                        if t >= 1:
                            # merged lag-1 masked add: m|rm halves at once
                            Mp = M_hist[t - 1]
                            lp1 = wkp.tile([128, 4, 128], f16, tag="lp1")
                            nc.vector.tensor_mul(lp1[:], Mp[:],
                                                 stM(t, 0, 128)
                                                 .rearrange("p (o n) -> p o n",
                                                            o=1)
                                                 .to_broadcast([128, 4, 128]))
                            nc.vector.tensor_add(S[:, :, 0:128],
                                                 S[:, :, 0:128], lp1[:])
                        zh_ps = scps.tile([128, 8, 64], f32, space="PSUM",
                                          tag="zh")
                        ez = stE(t, 512, 1024).rearrange(
                            "p (a b) -> p a b", a=4)
                        eh = stE(t, 1024, 1536).rearrange(
                            "p (a b) -> p a b", a=4)
                        for m in range(4):
                            nc.tensor.matmul(zh_ps[:, m, :],
                                             lhsT=ez[:, m, :],
                                             rhs=ident[:, 0:64],
                                             start=True, stop=False)
                            for k in range(4):
                                nc.tensor.matmul(zh_ps[:, m, :],
                                                 lhsT=wz[:, k, bass.ts(m, 128)],
                                                 rhs=S[:, k, 0:64],
                                                 start=False, stop=(k == 3))
                        for m in range(4):
                            nc.tensor.matmul(zh_ps[:, 4 + m, :],
                                             lhsT=eh[:, m, :],
                                             rhs=ident[:, 0:64],
                                             start=True, stop=False)
                            for k in range(4):
                                nc.tensor.matmul(zh_ps[:, 4 + m, :],
                                                 lhsT=wh[:, k, bass.ts(m, 128)],
                                                 rhs=S[:, k, 64:128],
                                                 start=False, stop=(k == 3))
                        z_t = wkp.tile([128, 4, 64], f16, tag="z")
                        nc.scalar.activation(z_t[:], zh_ps[:, 0:4, :],
                                             AF.Sigmoid)
                        th_t = wkp.tile([128, 4, 64], f16, tag="th")
                        nc.scalar.activation(th_t[:], zh_ps[:, 4:8, :],
                                             AF.Tanh)

                    # ---- wb(t-1): hslab + transpose + log write ----
                    if carry is not None:
                        tl = t - 1
                        pS = carry['S']
                        Mt = M_hist[tl]
                        GPE.tensor_add(hslab[:, :, bass.ts(tl + 1, 64)],
                                       pS[:, :, 128:192], Mt[:, :, 0:64])
                        if tl < DBG_T - 5:
                            # steps >= T-5 are never gathered (gather at t
                            # reads steps <= t-5) -> skip their log writes
                            tp_ = scps.tile([128, 4, 128], f16, space="PSUM",
                                            tag="tp")
                            for c in range(4):
                                nc.tensor.transpose(tp_[:, c, :], Mt[:, c, :],
                                                    ident[:])
                            stm = wkp.tile([128, 512], f16, tag="stm")
                            nc.scalar.activation(
                                stm[:], tp_[:].rearrange("p a b -> p (a b)"),
                                AF.Copy)
                            nc.scalar.dma_start(
                                out=mlog[tl * 128:(tl + 1) * 128, :],
                                in_=stm[:])

                    # prefetch AFTER wb so gather(t+PF) is emitted after
                    # the mlog write(t-1) it depends on (Tile program order)
                    if t < DBG_T and t + PF < DBG_T:
                        pend[t + PF] = prefetch(t + PF)

                    # ---- prep(t+1) + late lag adds (t+1) ----
                    # (emitted last: the A-matmul sits behind z/h on the PE
                    # queue, and the S-extract behind sigz/tanh on scalar)
                    if t < DBG_T and t + 1 < DBG_T:
                        st[t + 1] = prep(t + 1, pend.get(t + 1))
                    if t + 1 < DBG_T:
                        lag_late(t + 1, st[t + 1])

                    if t < DBG_T:
                        carry = dict(S=S, z=z_t, th=th_t)
                    else:
                        carry = None

            # ---- late constants (heads) on scalar queue ----
            wuwh = ld("wuwh", [128, 4, Hp], f16, eng=nc.scalar)
            wwwh = ld("wwwh", [128, 4, Hp], f16, eng=nc.scalar)
            wo = ld("wo", [128, 4, V], f16, eng=nc.scalar)
            us = ld("us", [128, 4], f16, eng=nc.scalar)
            twn = ld("twn", [128, 4, 128], f16, eng=nc.scalar)
            idt4 = ld("idt4", [128, 256], f16, eng=nc.scalar)
            qtg = ld("qtg", [128, RC], f32, eng=nc.scalar)
            qmask = ld("qmask", [128, RC], f32, eng=nc.scalar)
            ptgt = ld("ptgt", [128, RC], f32, eng=nc.scalar)
            pmask = ld("pmask", [128, RC], f32, eng=nc.scalar)
            iota_f = ld("iota", [128, V], f32, eng=nc.scalar)

            # ---------------- heads ----------------
            acc = cp.tile([128, 8], f32)
            nc.vector.memset(acc[:], 0.0)
            pl_sb = cp.tile([1, NCOL], f32)
            tl_all = cp.tile([128, RC], f32)
            cnt_all = cp.tile([128, RC], f32)
            se_all = cp.tile([128, RC], f32)
            ones32 = cp.tile([128, 1], f32)
            nc.vector.memset(ones32[:], 1.0)

            if not DBG_HEADS:
                nc.any.tensor_copy(acc[:, 0:1], hslab[:, 0, 0:1])
            if DBG_HEADS:
              # p-phase (all eu stream loads issued up front)
              with tc.tile_pool(name="php", bufs=2, space="PSUM") as php, \
                   tc.tile_pool(name="plps", bufs=2, space="PSUM") as plps, \
                   tc.tile_pool(name="pwk", bufs=2) as pwk, \
                   tc.tile_pool(name="eup", bufs=5) as eup:
                  eus_t = []
                  for cc in range(5):
                      eu = eup.tile([128, 4, 512], f16, tag="eu")
                      nc.sync.dma_start(
                          out=eu[:],
                          in_=D["eus"][:, cc * 2048:(cc + 1) * 2048]
                          .rearrange("p (c n) -> p c n", c=4))
                      eus_t.append(eu)
                  for cc in range(5):
                      cs = slice(cc * 512, (cc + 1) * 512)
                      eu = eus_t[cc]
                      # pp in 256-col halves (2 banks, bufs=2): the next
                      # block's matmuls no longer stall on this relu
                      ppre = pwk.tile([128, 4, 512], f16, tag="ppre")
                      for hh in range(2):
                          hs = slice(cc * 512 + hh * 256,
                                     cc * 512 + (hh + 1) * 256)
                          pp = php.tile([128, 4, 256], f32, space="PSUM",
                                        tag="pp")
                          for m in range(4):
                              for k in range(4):
                                  nc.tensor.matmul(
                                      pp[:, m, :],
                                      lhsT=wuwh[:, k, bass.ts(m, 128)],
                                      rhs=hslab[:, k, hs],
                                      start=(k == 0), stop=(k == 3))
                          nc.vector.tensor_add(
                              pp[:], pp[:],
                              eu[:, :, hh * 256:(hh + 1) * 256])
                          nc.scalar.activation(
                              ppre[:, :, hh * 256:(hh + 1) * 256], pp[:],
                              AF.Relu)
                      pl_ps = plps.tile([1, 512], f32, space="PSUM", tag="pl")
                      for k in range(4):
                          nc.tensor.matmul(pl_ps[:], lhsT=us[:, k:k + 1],
                                           rhs=ppre[:, k, :],
                                           start=(k == 0), stop=(k == 3))
                      nc.vector.tensor_copy(pl_sb[0:1, cs], pl_ps[:])

              # ---- p reductions (overlap with the q-phase) ----
              if DBG_FIN:
               scrp = cp.tile([128, RC], f32)
               nc.sync.dma_start(out=pl_dram[:], in_=pl_sb[:])
               pl_rm = cp.tile([128, RC], f32)
               nc.sync.dma_start(
                   out=pl_rm[:],
                   in_=pl_dram[0:1, :].rearrange("o (rc p) -> (o p) rc", p=128))
               # softplus(x) = relu(x) + ln(1 + exp(-|x|))
               ab = cp.tile([128, RC], f32)
               nc.scalar.activation(ab[:], pl_rm[:], AF.Abs)
               nc.scalar.activation(ab[:], ab[:], AF.Exp, scale=-1.0)
               nc.scalar.activation(ab[:], ab[:], AF.Ln, bias=1.0)
               rl = cp.tile([128, RC], f32)
               nc.scalar.activation(rl[:], pl_rm[:], AF.Relu)
               nc.vector.tensor_add(ab[:], ab[:], rl[:])
               nc.vector.tensor_tensor(out=scrp[:], in0=ab[:], in1=pmask[:],
                                       op=ALU.mult)
               nc.vector.tensor_reduce(out=acc[:, 1:2], in_=scrp[:], op=ALU.add,
                                       axis=mybir.AxisListType.X)
               nc.vector.tensor_tensor(out=scrp[:], in0=pl_rm[:], in1=ptgt[:],
                                       op=ALU.mult)
               nc.vector.tensor_reduce(out=acc[:, 4:5], in_=scrp[:], op=ALU.add,
                                       axis=mybir.AxisListType.X)
               # p match: (pl > 0) == ptgt
               gt = cp.tile([128, RC], f32)
               nc.vector.tensor_scalar(out=gt[:], in0=pl_rm[:], scalar1=0.0,
                                       scalar2=None, op0=ALU.is_gt)
               nc.vector.tensor_tensor(out=gt[:], in0=gt[:], in1=ptgt[:],
                                       op=ALU.is_equal)
               nc.vector.tensor_tensor(out=scrp[:], in0=gt[:], in1=pmask[:],
                                       op=ALU.mult)
               nc.vector.tensor_reduce(out=acc[:, 3:4], in_=scrp[:], op=ALU.add,
                                       axis=mybir.AxisListType.X)

              # q-phase
              if DBG_Q:
               with tc.tile_pool(name="qhp", bufs=1, space="PSUM") as qhp, \
                    tc.tile_pool(name="qlps", bufs=3, space="PSUM") as qlps, \
                    tc.tile_pool(name="qwk", bufs=2) as qwk:
                   for cc in range(5):
                       # qp in 256-col halves (2 PSUM banks) frees room for
                       # ql bufs=3 so the per-rr logit matmuls run ahead of
                       # the reductions without stalling the PE
                       qpre = qwk.tile([128, 4, 512], f16, tag="qpre")
                       for hh in range(2):
                           hs = slice(cc * 512 + hh * 256,
                                      cc * 512 + (hh + 1) * 256)
                           qp = qhp.tile([128, 4, 256], f32, space="PSUM",
                                         tag="qp")
                           for m in range(4):
                               nc.tensor.matmul(
                                   qp[:, m, :],
                                   lhsT=twn[:, m, :],
                                   rhs=idt4[:],
                                   start=True, stop=False)
                               for k in range(4):
                                   nc.tensor.matmul(
                                       qp[:, m, :],
                                       lhsT=wwwh[:, k, bass.ts(m, 128)],
                                       rhs=hslab[:, k, hs],
                                       start=False, stop=(k == 3))
                           nc.scalar.activation(
                               qpre[:, :, hh * 256:(hh + 1) * 256], qp[:],
                               AF.Relu)
                       for rr in range(4):
                           rc = cc * 4 + rr
                           ql = qlps.tile([128, V], f32, space="PSUM", tag="ql")
                           for k in range(4):
                               nc.tensor.matmul(ql[:, 0:512],
                                                lhsT=qpre[:, k, bass.ts(rr, 128)],
                                                rhs=wo[:, k, 0:512],
                                                start=(k == 0), stop=(k == 3))
                           for k in range(4):
                               nc.tensor.matmul(ql[:, 512:V],
                                                lhsT=qpre[:, k, bass.ts(rr, 128)],
                                                rhs=wo[:, k, 512:V],
                                                start=(k == 0), stop=(k == 3))
                           ohp = qwk.tile([128, V], f32, tag="ohp")
                           nc.vector.scalar_tensor_tensor(
                               out=ohp[:], in0=iota_f[:],
                               scalar=qtg[:, rc:rc + 1], in1=ql[:],
                               op0=ALU.is_equal, op1=ALU.mult,
                               accum_out=tl_all[:, rc:rc + 1])
                           esc = qwk.tile([128, V], f16, tag="esc")
                           nc.scalar.activation(esc[:], ql[:], AF.Exp,
                                                accum_out=se_all[:, rc:rc + 1])
                           # argmax match via Sign(tl - ql) on the scalar
                           # engine: sum = #lt - #gt over V-1 non-target
                           # logits (target gives Sign(0)=0), so
                           # argmax==tgt  <=>  sum == V-1.
                           sg = qwk.tile([128, V], f16, tag="sg")
                           nc.scalar.activation(sg[:], ql[:], AF.Sign,
                                                bias=tl_all[:, rc:rc + 1],
                                                scale=-1.0,
                                                accum_out=cnt_all[:, rc:rc + 1])

              # ---- final reductions ----
              if DBG_FIN:
               fin = cp.tile([128, RC], f32)
               # lse = ln(se) ; qterm = (lse - tl)*qmask summed
               nc.scalar.activation(fin[:], se_all[:], AF.Ln)
               nc.vector.tensor_sub(fin[:], fin[:], tl_all[:])
               scr = cp.tile([128, RC], f32)
               nc.vector.tensor_tensor(out=scr[:], in0=fin[:], in1=qmask[:],
                                       op=ALU.mult)
               nc.vector.tensor_reduce(out=acc[:, 0:1], in_=scr[:], op=ALU.add,
                                       axis=mybir.AxisListType.X)
               # q match: sum of Sign(tl - ql) == V-1
               nc.vector.tensor_scalar(out=fin[:], in0=cnt_all[:],
                                       scalar1=float(V - 1),
                                       scalar2=None, op0=ALU.is_equal)
               nc.vector.tensor_tensor(out=scr[:], in0=fin[:], in1=qmask[:],
                                       op=ALU.mult)
               nc.vector.tensor_reduce(out=acc[:, 2:3], in_=scr[:], op=ALU.add,
                                       axis=mybir.AxisListType.X)

               with tc.tile_pool(name="fps", bufs=1, space="PSUM") as fps:
                   fin_ps = fps.tile([1, 8], f32, space="PSUM")
                   nc.tensor.matmul(fin_ps[:], lhsT=ones32[:], rhs=acc[:],
                                    start=True, stop=True)
                   fin_sb = cp.tile([1, 8], f32)
                   nc.vector.tensor_copy(fin_sb[:], fin_ps[:])
                   nc.sync.dma_start(out=out_d[:], in_=fin_sb[:])

    nc.compile()
    return nc


_NC_CACHE = {}
LAST_EXEC_NS = None
LAST_RES = None


def kernel(**inputs):
    cores, nch, gcnts = _host_prep(inputs)
    key = (tuple(nch), tuple(gcnts), DBG_T, DBG_HEADS, DBG_Q, DBG_FIN, GPELEM)
    if key not in _NC_CACHE:
        _NC_CACHE[key] = _build_program(tuple(nch), tuple(gcnts))
    nc = _NC_CACHE[key]
    in_maps = [{k: np.ascontiguousarray(v) for k, v in cores[c].items()}
               for c in range(NC)]
    trace = os.environ.get("KERNEL_TRACE", "0") == "1"
    res = run_bass_kernel_spmd(nc, in_maps, core_ids=list(range(NC)),
                               trace=trace)
    global LAST_EXEC_NS, LAST_RES
    LAST_EXEC_NS = getattr(res, "exec_time_ns", None)
    LAST_RES = res
    total = np.zeros(8, np.float64)
    for r in res.results:
        total += np.asarray(r["out"], np.float64).reshape(-1)
    q_loss = total[0] / B
    p_loss = (total[1] - total[4]) / B
    q_acc = total[2] / 10240.0
    p_acc = total[3] / (39 * B)
    return np.array([q_loss, p_loss, q_acc, p_acc], np.float32)


if __name__ == "__main__":
    pass


# revision 31
# speedup vs baseline: 1.2737x; 1.2737x over previous
"""Trainium2 Bass kernel for the DGL-JTNN tree decoder (nn_DGLJTNNDecoder).

Strategy: pure data-parallel over the 512 trees, 64 trees per NeuronCore.

v3 redesign (vs. v2):
  - DFS lag structure exploited: pred-edge lags are always odd (1,3,5,...)
    and node-inbox lags always even (2,4,...).  Masked SBUF adds now cover
    pred lag-1/lag-3 and node lag-2/lag-4 (the old lag-2 pred / lag-1 node
    masks never fired); the DRAM step-log gather only covers lag>=5, so it
    can be issued 4 iterations early (PF=4) and round-robins over 2 SWDGE
    queues -> fully hidden (v2 stalled ~5us/step on a single-queue gather).
  - Per-step streams (Er/Ez/Eh embedding rows, masks, A-routing chunks) are
    packed into one DRAM stream loaded 4 steps per DMA: ~32 descriptors per
    step instead of ~256 (the HWDGE queues are descriptor-rate-bound).
  - Er/Ez/Eh ship TREE-major and are preloaded into PSUM by identity
    matmuls at the head of each accumulation group (PE-ordered, race-free),
    removing three vector adds per step from the critical path.
  - q-head: tree_vec projection preloaded by identity matmul (was a
    broadcast vector add per 256-col block); the softmax max-reduce is
    dropped (|logits| < 1, exp is safe); argmax match is computed as
    "count of logits > target logit" via scalar_tensor_tensor accum.
Losses/accuracies reduce to 8 partial sums per core, combined on the host.
"""

import os
import numpy as np

import concourse.bass as bass
import concourse.bacc as bacc
import concourse.mybir as mybir
import concourse.tile as tile
from concourse.library_config import mlp as _mlp_lib
from concourse.bass_utils import run_bass_kernel_spmd

f16 = mybir.dt.float16
f32 = mybir.dt.float32
i16 = mybir.dt.int16
AF = mybir.ActivationFunctionType
ALU = mybir.AluOpType

# problem constants (hardcoded per contract)
B, N, H, L, V = 512, 20, 450, 56, 780
T = 2 * (N - 1)            # 38 steps
NC = 8                     # cores
C = B // NC                # 64 trees/core
Hp = 512                   # padded hidden
NBLK = 40                  # head col blocks (39 real + 1 pad) -> 2560 cols
NCOL = NBLK * C            # 2560
RC = NCOL // 128           # 20 row chunks
P, Dn = 3, 4
LOG_ROWS = T * 128         # state log rows (step t -> rows t*128 : +128)
SG = 4                     # steps per stream-group DMA
PF = 6                     # gather prefetch depth (gather t reads steps <= t-7)
NSWQ = 2                   # SWDGE queues for gathers (round-robin)

DBG_T = int(os.environ.get("KDBG_T", T))
DBG_HEADS = os.environ.get("KDBG_HEADS", "1") == "1"
DBG_Q = os.environ.get("KDBG_Q", "1") == "1"
DBG_FIN = os.environ.get("KDBG_FIN", "1") == "1"
GPELEM = os.environ.get("KDBG_GPELEM", "0") == "1"
GFULL = 0                  # extra gather steps forced to full 128-row counts


def _wrap_idx(idx):
    """[n*16] flat gather order -> [16, n] wrapped, replicated to 128 rows."""
    idx = np.asarray(idx, np.int16)
    n = idx.shape[0] // 16
    return np.tile(idx.reshape(n, 16).T, (8, 1))    # [128, n]


def _host_prep(inputs):
    inp = {k: np.asarray(v) for k, v in inputs.items()}
    (tree_vec, emb, Wz, bz, Wh, bh, Wr, Ur, br, Ww, bw, Uw, bu, Wo, bo,
     Us, bs) = (inp[k] for k in
                ['tree_vec', 'emb', 'Wz', 'bz', 'Wh', 'bh', 'Wr', 'Ur', 'br',
                 'Ww', 'bw', 'Uw', 'bu', 'Wo', 'bo', 'Us', 'bs'])
    wid, root_ids = inp['wid'], inp['root_ids']
    edge_src, edge_dst = inp['edge_src'], inp['edge_dst']
    edge_pred, node_in = inp['edge_pred'], inp['node_in']
    step_eid, step_v = inp['step_eid'], inp['step_v']
    q_rows, q_tgt, p_tgt = inp['q_rows'], inp['q_tgt'], inp['p_tgt']
    n_edges = edge_src.shape[0]

    def padHp(M, axis):
        pads = [(0, 0)] * M.ndim
        pads[axis] = (0, Hp - M.shape[axis])
        return np.pad(M, pads)

    def h16(x):
        return np.ascontiguousarray(x.astype(np.float16))

    # ---- folded tables (fp32 math) ----
    Ez = padHp(emb @ Wz[:H] + bz, 1)
    Eh = padHp(emb @ Wh[:H] + bh, 1)
    Er = padHp(emb @ Wr + br, 1)
    Eu = padHp(emb @ Uw[:H] + bu, 1)
    Eu[:, 511] = 1.0                      # bias-injection channel for p-head

    def packW(Wm):                         # [512,512] -> [128, 4*512]
        return h16(Wm.reshape(4, 128, Hp).transpose(1, 0, 2).reshape(128, 4 * Hp))

    wz_h = packW(padHp(padHp(Wz[H:], 0), 1))
    wh_h = packW(padHp(padHp(Wh[H:], 0), 1))
    wu_h = packW(padHp(padHp(Ur, 0), 1))
    wuwh_h = packW(padHp(padHp(Uw[H:2 * H], 0), 1))
    wwwh_h = packW(padHp(padHp(Ww[:H], 0), 1))
    Wop = padHp(Wo, 0)
    Wop[511, :] = bo                      # bias row (qrelu[511]==1)
    wo_h = h16(Wop.reshape(4, 128, V).transpose(1, 0, 2).reshape(128, 4 * V))
    Usp = padHp(Us, 0)
    Usp[511, 0] = bs[0]                   # bias row (prelu[511]==1)
    us_h = h16(Usp.reshape(4, 128).T)     # [128, 4]

    # tree_vec with ones column (bias channel), host-side projections
    tvpad = np.zeros((B, 64), np.float32)
    tvpad[:, :L] = tree_vec
    tvpad[:, L] = 1.0
    WuL = np.zeros((64, Hp), np.float32)
    WuL[:L] = padHp(Uw[2 * H:], 1)
    WwL = np.zeros((64, Hp), np.float32)
    WwL[:L] = padHp(Ww[H:], 1)
    WwL[L, :H] = bw                       # ones channel -> +bw
    WwL[L, 511] = 1.0                     # makes qrelu[511]==1

    estep = np.full(n_edges, -1, np.int64)
    for t in range(T):
        for b in range(B):
            estep[step_eid[t, b]] = t

    # ---- per-core graph metadata ----
    # masks layout per step: P1m 0:64 | P1rm 64:128 | P3 128:256 (m|rm) |
    #                        N2 256:320 | N4 320:384 | P5 384:512 (m|rm) |
    #                        N6 512:576
    raw = []
    for core in range(NC):
        trees = np.arange(core * C, (core + 1) * C)
        masks = np.zeros((T, 576), np.float16)
        grows = [dict() for _ in range(T)]            # log row -> set(out cols)
        for t in range(T):
            for j, b in enumerate(trees):
                e = step_eid[t, b]
                v = step_v[t, b]
                for p in range(P):
                    pe = edge_pred[e, p]
                    if pe >= n_edges:
                        continue
                    tp = estep[pe]
                    if tp > t:
                        continue
                    lag = t - tp
                    if lag == 1:
                        masks[t, j] = 1.0; masks[t, 64 + j] = 1.0
                    elif lag == 3:
                        masks[t, 128 + j] = 1.0; masks[t, 192 + j] = 1.0
                    elif lag == 5:
                        masks[t, 384 + j] = 1.0; masks[t, 448 + j] = 1.0
                    else:
                        grows[t].setdefault(tp * 128 + j, set()).add(j)
                        grows[t].setdefault(tp * 128 + 64 + j, set()).add(64 + j)
                for p in range(Dn):
                    ie = node_in[v, p]
                    if ie >= n_edges or ie == e:
                        continue
                    ti = estep[ie]
                    if ti > t:
                        continue
                    lag = t - ti
                    if lag == 2:
                        masks[t, 256 + j] = 1.0
                    elif lag == 4:
                        masks[t, 320 + j] = 1.0
                    elif lag == 6:
                        masks[t, 512 + j] = 1.0
                    else:
                        grows[t].setdefault(ti * 128 + j, set()).add(128 + j)

        # head Eu stream column word ids
        widrow = np.zeros(NCOL, np.int64)
        widrow[:C] = wid[root_ids[trees]]
        for t in range(T):
            widrow[(t + 1) * 64:(t + 2) * 64] = wid[step_v[t, trees]]

        ws = wid[edge_src[step_eid[:, trees]]]        # [T, C]
        wd = wid[edge_dst[step_eid[:, trees]]]

        # q/p loss tables, row-major [128, 20]
        qmask = np.zeros((128, RC), np.float32)
        qtg = np.zeros((128, RC), np.float32)
        ptgt = np.zeros((128, RC), np.float32)
        pmask = np.zeros((128, RC), np.float32)
        for i in range(q_rows.shape[0]):
            g = int(q_rows[i])
            k, b = g // B, g % B
            if core * C <= b < (core + 1) * C:
                l = k * C + (b - core * C)
                qmask[l % 128, l // 128] = 1.0
                qtg[l % 128, l // 128] = float(q_tgt[i])
        for l in range(39 * C):
            k, j = l // C, l % C
            g = k * B + core * C + j
            ptgt[l % 128, l // 128] = float(p_tgt[g])
            pmask[l % 128, l // 128] = 1.0

        raw.append(dict(trees=trees, masks=masks, grows=grows, widrow=widrow,
                        ws=ws, wd=wd, qtg=qtg, qmask=qmask, ptgt=ptgt,
                        pmask=pmask))

    # shared per-step chunk counts (program structure; max over cores)
    nch = tuple(max((len(r['grows'][t]) + 127) // 128 for r in raw)
                for t in range(T))
    AW = sum(nch)
    aoffs = np.concatenate([[0], np.cumsum(nch)]).astype(np.int64)
    # shared per-step gather counts (compile-time constants; max over cores,
    # rounded up to 16).  Per-core idx arrays pad real rows up to the shared
    # count with valid row 0, then -1 (ucode: count == #non-negative idxs).
    gstep0 = next((t for t in range(T) if nch[t]), T)
    gcnts = []
    for t in range(T):
        if nch[t] == 0:
            gcnts.append(0)
        elif t < gstep0 + GFULL:
            gcnts.append(nch[t] * 128)
        else:
            mx = max(len(r['grows'][t]) for r in raw)
            gcnts.append(min(max(16, (mx + 15) // 16 * 16), nch[t] * 128))
    gcnts = tuple(gcnts)
    NCHMAX = max(nch) if nch else 0
    # stream E-part (64 tree partitions): er_tm 512 | ez_tm 512 | eh_tm 512
    # stream M-part (128 partitions): masks 384 | at NCHMAX*192
    WSM = 576 + 192 * NCHMAX
    NG = (T + SG - 1) // SG

    # ---- per-core input tensors ----
    cores = []
    for core in range(NC):
        r = raw[core]
        gidx_w = np.zeros((128, max(AW, 1) * 8), np.int16)
        strmE = np.zeros((64, NG * SG * 1536), np.float16)
        strmM = np.zeros((128, NG * SG * WSM), np.float16)
        for t in range(T):
            n = nch[t]
            be = t * 1536
            strmE[:, be:be + 512] = Er[r['wd'][t]]
            strmE[:, be + 512:be + 1024] = Ez[r['ws'][t]]
            strmE[:, be + 1024:be + 1536] = Eh[r['ws'][t]]
            bm = t * WSM
            strmM[:, bm:bm + 576] = np.broadcast_to(r['masks'][t], (128, 576))
            if n == 0:
                continue
            rows = sorted(r['grows'][t].keys())
            cnt16 = gcnts[t]
            # ucode contract: count reg == #non-negative idxs.  Pad the
            # real rows up to the shared cnt16 with valid row 0, then -1.
            idx = np.full(n * 128, -1, np.int64)
            idx[:cnt16] = 0
            idx[:len(rows)] = rows
            gidx_w[:, aoffs[t] * 8:(aoffs[t] + n) * 8] = _wrap_idx(idx)
            A = np.zeros((n * 128, 192), np.float16)
            for i, row in enumerate(rows):
                for col in r['grows'][t][row]:
                    A[i, col] = 1.0
            # gathered out[p, c, :] = log[idx[c*128+p]] -> A chunk c row p
            for c in range(n):
                strmM[:, bm + 576 + c * 192:bm + 576 + (c + 1) * 192] = \
                    A[c * 128:(c + 1) * 128]

        # tree_vec projections
        tv = tvpad[r['trees']]                         # [C, 64]
        tun = (tv @ WuL).reshape(C, 4, 128).transpose(2, 1, 0)   # feature-major
        twn_tm = np.zeros((128, Hp), np.float32)
        twn_tm[:C] = tv @ WwL                          # tree-major [128, 512]

        # idt4: 64-identity tiled 4x horizontally [128, 256]
        idt4 = np.zeros((128, 256), np.float16)
        for j in range(C):
            for k4 in range(4):
                idt4[j, k4 * 64 + j] = 1.0

        # eus stream: per cc [128, 4, 512] = Eu[widrow]+tun folded
        eus = np.zeros((128, 5 * 2048), np.float16)
        for cc in range(5):
            eu = Eu[r['widrow'][cc * 512:(cc + 1) * 512]]
            eu = eu.reshape(512, 4, 128).transpose(2, 1, 0)   # [128, 4, 512]
            eu = eu + np.tile(tun, (1, 1, 8))
            eus[:, cc * 2048:(cc + 1) * 2048] = eu.reshape(128, 2048)

        cores.append(dict(
            wz=wz_h, wh=wh_h, wu=wu_h, wuwh=wuwh_h, wwwh=wwwh_h,
            wo=wo_h, us=us_h,
            strmE=strmE, strmM=strmM, gidx=gidx_w,
            eus=eus,
            twn=np.ascontiguousarray(twn_tm.astype(np.float16)),
            idt4=idt4,
            qtg=r['qtg'], qmask=r['qmask'], ptgt=r['ptgt'], pmask=r['pmask'],
            iota=np.broadcast_to(np.arange(V, dtype=np.float32), (128, V)).copy(),
        ))
    return cores, nch, gcnts


def _build_program(nch, gcnts):
    AW = sum(nch)
    NCHMAX = max(nch) if nch else 0
    WSM = 576 + 192 * NCHMAX
    NG = (T + SG - 1) // SG
    aoffs = np.concatenate([[0], np.cumsum(np.asarray(nch, np.int64))])
    nc = bacc.Bacc("TRN2", debug=False, num_swdge_queues=NSWQ)
    GPE = nc.gpsimd if GPELEM else nc.vector
    i32 = mybir.dt.int32

    D = {}
    def di(name, shape, dt):
        D[name] = nc.dram_tensor(name, shape, dt, kind="ExternalInput")
        return D[name]

    for w in ["wz", "wh", "wu", "wuwh", "wwwh"]:
        di(w, [128, 4 * Hp], f16)
    di("wo", [128, 4 * V], f16)
    di("us", [128, 4], f16)
    di("strmE", [64, NG * SG * 1536], f16)
    di("strmM", [128, NG * SG * WSM], f16)
    di("gidx", [128, max(AW, 1) * 8], i16)
    di("eus", [128, 5 * 2048], f16)
    di("twn", [128, Hp], f16)
    di("idt4", [128, 256], f16)
    for x in ["qtg", "qmask", "ptgt", "pmask"]:
        di(x, [128, RC], f32)
    di("iota", [128, V], f32)
    out_d = nc.dram_tensor("out", [1, 8], f32, kind="ExternalOutput")

    mlog = nc.dram_tensor("mlog", [LOG_ROWS, Hp], f16, kind="Internal")
    pl_dram = nc.dram_tensor("pl_scratch", [1, NCOL], f32, kind="Internal")

    with tile.TileContext(nc) as tc:
        with tc.tile_pool(name="const", bufs=1) as cp:
            nc.gpsimd.load_library(_mlp_lib)
            # ---- early constants (scan) on sync queue ----
            def ld(name, shape, dt, eng=None):
                t_ = cp.tile(shape, dt, tag=name)
                eng = eng or nc.sync
                eng.dma_start(out=t_[:], in_=D[name][:].rearrange(
                    "p (a b) -> p a b", a=shape[1]) if len(shape) == 3 else D[name][:])
                return t_
            wz = ld("wz", [128, 4, Hp], f16)
            wh = ld("wh", [128, 4, Hp], f16)
            wu = ld("wu", [128, 4, Hp], f16)
            gidx = ld("gidx", [128, max(AW, 1), 8], i16) if AW else None
            rg = {c: nc.gpsimd.to_reg(c) for c in sorted(set(gcnts)) if c}

            ident = cp.tile([128, 128], f16)
            from concourse.masks import make_identity
            make_identity(nc, ident[:])

            # h slab (feature-major), zeroed (roots + pad cols)
            hslab = cp.tile([128, 4, NCOL], f16)
            nc.vector.memset(hslab[:], 0.0)

            # persistent E-stream tiles (manual 3-rotation): partitions
            # 64:128 are never DMA'd -- zero them once so the identity-rhs
            # matmuls contract finite*0 (stale NaN*0 would poison PSUM)
            esl = []
            for i in range(3):
                e_ = cp.tile([128, SG, 1536], f16, tag=f"esl{i}")
                nc.vector.memset(e_[64:128, :, :], 0.0)
                esl.append(e_)

            # ---------------- scan ----------------
            # Per loop iteration t we emit:
            #   stream(group):  4-step stream DMA at group boundaries
            #   tail(t-1):     d-chain -> Mt, r-gate (er-preload mm), rm
            #   prep(t+1):     A-matmul -> S_ps, S/hp extraction
            #   lagL(t+1):     P3/N2/N4 masked adds (need M(t-1) at most)
            #   head(t):       P1 adds + z/h matmul groups (ez/eh preload)
            #   wb(t-1):       hslab, transpose, log write (off-chain)
            #   prefetch(t+PF): gather (round-robin SWDGE queues)
            with tc.tile_pool(name="strm", bufs=3) as sp, \
                 tc.tile_pool(name="g", bufs=8) as gp, \
                 tc.tile_pool(name="wk", bufs=4) as wkp, \
                 tc.tile_pool(name="mrm", bufs=8) as mrmp, \
                 tc.tile_pool(name="sps", bufs=1, space="PSUM") as sps, \
                 tc.tile_pool(name="scps", bufs=2, space="PSUM") as scps:
                M_hist = []

                def load_group(g):
                    if g * SG >= DBG_T or g >= NG:
                        return None
                    e_ = esl[g % 3]
                    nc.sync.dma_start(
                        out=e_[0:64, :, :],
                        in_=D["strmE"][:, g * SG * 1536:(g + 1) * SG * 1536]
                        .rearrange("p (s w) -> p s w", s=SG))
                    m_ = sp.tile([128, SG, WSM], f16, tag="strm")
                    nc.sync.dma_start(
                        out=m_[:],
                        in_=D["strmM"][:, g * SG * WSM:(g + 1) * SG * WSM]
                        .rearrange("p (s w) -> p s w", s=SG))
                    return m_

                sgrp = {}
                for g0 in range(2):
                    sgrp[g0] = load_group(g0)

                def stE(t, a, b):
                    return esl[(t // SG) % 3][:, t % SG, a:b]

                def stM(t, a, b):
                    return sgrp[t // SG][:, t % SG, a:b]

                def prefetch(tf):
                    """gather for step tf (reads log steps <= tf-5)."""
                    if tf >= DBG_T or not nch[tf]:
                        return None
                    ncht = nch[tf]
                    g = gp.tile([128, ncht, Hp], f16, tag="g")
                    ao = int(aoffs[tf])
                    nc.gpsimd.dma_gather(
                        g[:], mlog[0:max(tf - 6, 1) * 128, :],
                        gidx[:, ao:ao + ncht, :].rearrange(
                            "p a b -> p (a b)"),
                        ncht * 128, rg[gcnts[tf]], Hp,
                        transpose=False, queue_num=tf % NSWQ)
                    return g

                def prep(ts_, g):
                    """A-matmul + merged S|hp extraction for step ts_.
                    S layout: [:, :, 0:64] s, [64:128] arm, [128:192] hp."""
                    if ts_ >= DBG_T:
                        return None
                    ncht = nch[ts_]
                    S = wkp.tile([128, 4, 192], f16, tag="S")
                    if ncht:
                        at = stM(ts_, 576, 576 + ncht * 192).rearrange(
                            "p (c n) -> p c n", c=ncht)
                        S_ps = sps.tile([128, 4, 256], f32, space="PSUM",
                                        tag="sps")
                        # contract only the gathered rows: chunk c covers
                        # gathered rows [c*128, c*128+kc) -- rows past the
                        # per-step count are never written (stale SBUF)
                        cnt = gcnts[ts_]
                        for m in range(4):
                            for c in range(ncht):
                                kc = min(cnt - c * 128, 128)
                                if kc <= 0:
                                    continue
                                nc.tensor.matmul(S_ps[:, m, 0:192],
                                                 lhsT=g[0:kc, c,
                                                        bass.ts(m, 128)],
                                                 rhs=at[0:kc, c, :],
                                                 start=(c == 0),
                                                 stop=(c == ncht - 1 or
                                                       cnt - c * 128 <= 128))
                        nc.scalar.activation(S[:], S_ps[:, :, 0:192], AF.Copy)
                    else:
                        nc.vector.memset(S[:], 0.0)
                    return {'S': S}

                def lag_late(ts_, cur):
                    """P3/N2/N4 masked adds for step ts_ (need M(ts_-2))."""
                    if cur is None:
                        return
                    S = cur['S']
                    if ts_ >= 3:
                        Mp = M_hist[ts_ - 3]
                        lg3 = wkp.tile([128, 4, 128], f16, tag="lg3")
                        nc.vector.tensor_mul(lg3[:], Mp[:],
                                             stM(ts_, 128, 256)
                                             .rearrange("p (o n) -> p o n", o=1)
                                             .to_broadcast([128, 4, 128]))
                        nc.vector.tensor_add(S[:, :, 0:128], S[:, :, 0:128],
                                             lg3[:])
                    if ts_ >= 2:
                        Mp = M_hist[ts_ - 2]
                        ln2 = wkp.tile([128, 4, 64], f16, tag="ln2")
                        GPE.tensor_mul(ln2[:], Mp[:, :, 0:64],
                                       stM(ts_, 256, 320)
                                       .rearrange("p (o n) -> p o n", o=1)
                                       .to_broadcast([128, 4, 64]))
                        GPE.tensor_add(S[:, :, 128:192], S[:, :, 128:192],
                                       ln2[:])
                    if ts_ >= 4:
                        Mp = M_hist[ts_ - 4]
                        ln4 = wkp.tile([128, 4, 64], f16, tag="ln4")
                        GPE.tensor_mul(ln4[:], Mp[:, :, 0:64],
                                       stM(ts_, 320, 384)
                                       .rearrange("p (o n) -> p o n", o=1)
                                       .to_broadcast([128, 4, 64]))
                        GPE.tensor_add(S[:, :, 128:192], S[:, :, 128:192],
                                       ln4[:])
                    if ts_ >= 5:
                        Mp = M_hist[ts_ - 5]
                        lg5 = wkp.tile([128, 4, 128], f16, tag="lg5")
                        nc.vector.tensor_mul(lg5[:], Mp[:],
                                             stM(ts_, 384, 512)
                                             .rearrange("p (o n) -> p o n", o=1)
                                             .to_broadcast([128, 4, 128]))
                        nc.vector.tensor_add(S[:, :, 0:128], S[:, :, 0:128],
                                             lg5[:])
                    if ts_ >= 6:
                        Mp = M_hist[ts_ - 6]
                        ln6 = wkp.tile([128, 4, 64], f16, tag="ln6")
                        GPE.tensor_mul(ln6[:], Mp[:, :, 0:64],
                                       stM(ts_, 512, 576)
                                       .rearrange("p (o n) -> p o n", o=1)
                                       .to_broadcast([128, 4, 64]))
                        GPE.tensor_add(S[:, :, 128:192], S[:, :, 128:192],
                                       ln6[:])

                pend = {}
                for tf in range(min(PF, DBG_T)):
                    pend[tf] = prefetch(tf)
                st = {0: prep(0, pend.get(0))}
                lag_late(0, st[0])
                carry = None
                for t in range(DBG_T + 1):
                    if t < DBG_T:
                        cur = st.pop(t)
                    # stream prefetch (t%SG==1: the evicted group's last
                    # reader was tail(t-1) of the previous iteration)
                    if t < DBG_T and t % SG == 1:
                        g2 = t // SG + 2
                        if g2 not in sgrp:
                            sgrp[g2] = load_group(g2)

                    # ---- tail(t-1): d-chain -> M, r-gate, rm ----
                    if carry is not None:
                        tl = t - 1
                        pS, z_t, th_t = carry['S'], carry['z'], carry['th']
                        Mt = mrmp.tile([128, 4, 128], f16, tag="M")
                        d_t = wkp.tile([128, 4, 64], f16, tag="d")
                        nc.vector.tensor_sub(d_t[:], th_t[:], pS[:, :, 0:64])
                        nc.vector.tensor_mul(d_t[:], z_t[:], d_t[:])
                        nc.vector.tensor_add(Mt[:, :, 0:64], d_t[:],
                                             pS[:, :, 0:64])
                        r_ps = scps.tile([128, 4, 64], f32, space="PSUM",
                                         tag="r")
                        er = stE(tl, 0, 512).rearrange(
                            "p (a b) -> p a b", a=4)
                        for m in range(4):
                            nc.tensor.matmul(r_ps[:, m, :],
                                             lhsT=er[:, m, :],
                                             rhs=ident[:, 0:64],
                                             start=True, stop=False)
                            for k in range(4):
                                nc.tensor.matmul(r_ps[:, m, :],
                                                 lhsT=wu[:, k, bass.ts(m, 128)],
                                                 rhs=Mt[:, k, 0:64],
                                                 start=False, stop=(k == 3))
                        r_t = wkp.tile([128, 4, 64], f16, tag="rt")
                        nc.scalar.activation(r_t[:], r_ps[:], AF.Sigmoid)
                        nc.vector.tensor_mul(Mt[:, :, 64:128], r_t[:],
                                             Mt[:, :, 0:64])
                        M_hist.append(Mt)

                    # ---- prep(t+1) + late lag adds (t+1) ----
                    if t < DBG_T and t + 1 < DBG_T:
                        st[t + 1] = prep(t + 1, pend.get(t + 1))
                    if t + 1 < DBG_T:
                        lag_late(t + 1, st[t + 1])

                    # ---- head(t): P1 add + z/h groups + activations ----
                    if t < DBG_T:
                        S = cur['S']
                        if t >= 1:
                            # merged lag-1 masked add: m|rm halves at once
                            Mp = M_hist[t - 1]
                            lp1 = wkp.tile([128, 4, 128], f16, tag="lp1")
                            nc.vector.tensor_mul(lp1[:], Mp[:],
                                                 stM(t, 0, 128)
                                                 .rearrange("p (o n) -> p o n",
                                                            o=1)
                                                 .to_broadcast([128, 4, 128]))
                            nc.vector.tensor_add(S[:, :, 0:128],
                                                 S[:, :, 0:128], lp1[:])
                        zh_ps = scps.tile([128, 8, 64], f32, space="PSUM",
                                          tag="zh")
                        ez = stE(t, 512, 1024).rearrange(
                            "p (a b) -> p a b", a=4)
                        eh = stE(t, 1024, 1536).rearrange(
                            "p (a b) -> p a b", a=4)
                        for m in range(4):
                            nc.tensor.matmul(zh_ps[:, m, :],
                                             lhsT=ez[:, m, :],
                                             rhs=ident[:, 0:64],
                                             start=True, stop=False)
                            for k in range(4):
                                nc.tensor.matmul(zh_ps[:, m, :],
                                                 lhsT=wz[:, k, bass.ts(m, 128)],
                                                 rhs=S[:, k, 0:64],
                                                 start=False, stop=(k == 3))
                        for m in range(4):
                            nc.tensor.matmul(zh_ps[:, 4 + m, :],
                                             lhsT=eh[:, m, :],
                                             rhs=ident[:, 0:64],
                                             start=True, stop=False)
                            for k in range(4):
                                nc.tensor.matmul(zh_ps[:, 4 + m, :],
                                                 lhsT=wh[:, k, bass.ts(m, 128)],
                                                 rhs=S[:, k, 64:128],
                                                 start=False, stop=(k == 3))
                        z_t = wkp.tile([128, 4, 64], f16, tag="z")
                        nc.scalar.activation(z_t[:], zh_ps[:, 0:4, :],
                                             AF.Sigmoid)
                        th_t = wkp.tile([128, 4, 64], f16, tag="th")
                        nc.scalar.activation(th_t[:], zh_ps[:, 4:8, :],
                                             AF.Tanh)

                    # ---- wb(t-1): hslab + transpose + log write ----
                    if carry is not None:
                        tl = t - 1
                        pS = carry['S']
                        Mt = M_hist[tl]
                        GPE.tensor_add(hslab[:, :, bass.ts(tl + 1, 64)],
                                       pS[:, :, 128:192], Mt[:, :, 0:64])
                        if tl < DBG_T - 7:
                            # steps >= T-7 are never gathered (gather at t
                            # reads steps <= t-7) -> skip their log writes
                            tp_ = scps.tile([128, 4, 128], f16, space="PSUM",
                                            tag="tp")
                            for c in range(4):
                                nc.tensor.transpose(tp_[:, c, :], Mt[:, c, :],
                                                    ident[:])
                            stm = wkp.tile([128, 512], f16, tag="stm")
                            nc.scalar.activation(
                                stm[:], tp_[:].rearrange("p a b -> p (a b)"),
                                AF.Copy)
                            nc.scalar.dma_start(
                                out=mlog[tl * 128:(tl + 1) * 128, :],
                                in_=stm[:])

                    # prefetch AFTER wb so gather(t+PF) is emitted after
                    # the mlog write(t-1) it depends on (Tile program order)
                    if t < DBG_T and t + PF < DBG_T:
                        pend[t + PF] = prefetch(t + PF)

                    if t < DBG_T:
                        carry = dict(S=S, z=z_t, th=th_t)
                    else:
                        carry = None

            # ---- late constants (heads) on scalar queue ----
            wuwh = ld("wuwh", [128, 4, Hp], f16, eng=nc.scalar)
            wwwh = ld("wwwh", [128, 4, Hp], f16, eng=nc.scalar)
            wo = ld("wo", [128, 4, V], f16, eng=nc.scalar)
            us = ld("us", [128, 4], f16, eng=nc.scalar)
            twn = ld("twn", [128, 4, 128], f16, eng=nc.scalar)
            idt4 = ld("idt4", [128, 256], f16, eng=nc.scalar)
            qtg = ld("qtg", [128, RC], f32, eng=nc.scalar)
            qmask = ld("qmask", [128, RC], f32, eng=nc.scalar)
            ptgt = ld("ptgt", [128, RC], f32, eng=nc.scalar)
            pmask = ld("pmask", [128, RC], f32, eng=nc.scalar)
            iota_f = ld("iota", [128, V], f32, eng=nc.scalar)

            # ---------------- heads ----------------
            acc = cp.tile([128, 8], f32)
            nc.vector.memset(acc[:], 0.0)
            pl_sb = cp.tile([1, NCOL], f32)
            tl_all = cp.tile([128, RC], f32)
            cnt_all = cp.tile([128, RC], f32)
            se_all = cp.tile([128, RC], f32)
            ones32 = cp.tile([128, 1], f32)
            nc.vector.memset(ones32[:], 1.0)

            if not DBG_HEADS:
                nc.any.tensor_copy(acc[:, 0:1], hslab[:, 0, 0:1])
            if DBG_HEADS:
              # p-phase (all eu stream loads issued up front)
              with tc.tile_pool(name="php", bufs=2, space="PSUM") as php, \
                   tc.tile_pool(name="plps", bufs=2, space="PSUM") as plps, \
                   tc.tile_pool(name="pwk", bufs=2) as pwk, \
                   tc.tile_pool(name="eup", bufs=5) as eup:
                  eus_t = []
                  for cc in range(5):
                      eu = eup.tile([128, 4, 512], f16, tag="eu")
                      nc.sync.dma_start(
                          out=eu[:],
                          in_=D["eus"][:, cc * 2048:(cc + 1) * 2048]
                          .rearrange("p (c n) -> p c n", c=4))
                      eus_t.append(eu)
                  for cc in range(5):
                      cs = slice(cc * 512, (cc + 1) * 512)
                      eu = eus_t[cc]
                      # pp in 256-col halves (2 banks, bufs=2): the next
                      # block's matmuls no longer stall on this relu
                      ppre = pwk.tile([128, 4, 512], f16, tag="ppre")
                      for hh in range(2):
                          hs = slice(cc * 512 + hh * 256,
                                     cc * 512 + (hh + 1) * 256)
                          pp = php.tile([128, 4, 256], f32, space="PSUM",
                                        tag="pp")
                          for m in range(4):
                              for k in range(4):
                                  nc.tensor.matmul(
                                      pp[:, m, :],
                                      lhsT=wuwh[:, k, bass.ts(m, 128)],
                                      rhs=hslab[:, k, hs],
                                      start=(k == 0), stop=(k == 3))
                          nc.vector.tensor_add(
                              pp[:], pp[:],
                              eu[:, :, hh * 256:(hh + 1) * 256])
                          nc.scalar.activation(
                              ppre[:, :, hh * 256:(hh + 1) * 256], pp[:],
                              AF.Relu)
                      pl_ps = plps.tile([1, 512], f32, space="PSUM", tag="pl")
                      for k in range(4):
                          nc.tensor.matmul(pl_ps[:], lhsT=us[:, k:k + 1],
                                           rhs=ppre[:, k, :],
                                           start=(k == 0), stop=(k == 3))
                      nc.vector.tensor_copy(pl_sb[0:1, cs], pl_ps[:])

              # ---- p reductions (overlap with the q-phase) ----
              if DBG_FIN:
               scrp = cp.tile([128, RC], f32)
               nc.sync.dma_start(out=pl_dram[:], in_=pl_sb[:])
               pl_rm = cp.tile([128, RC], f32)
               nc.sync.dma_start(
                   out=pl_rm[:],
                   in_=pl_dram[0:1, :].rearrange("o (rc p) -> (o p) rc", p=128))
               # softplus(x) = relu(x) + ln(1 + exp(-|x|))
               ab = cp.tile([128, RC], f32)
               nc.scalar.activation(ab[:], pl_rm[:], AF.Abs)
               nc.scalar.activation(ab[:], ab[:], AF.Exp, scale=-1.0)
               nc.scalar.activation(ab[:], ab[:], AF.Ln, bias=1.0)
               rl = cp.tile([128, RC], f32)
               nc.scalar.activation(rl[:], pl_rm[:], AF.Relu)
               nc.vector.tensor_add(ab[:], ab[:], rl[:])
               nc.vector.tensor_tensor(out=scrp[:], in0=ab[:], in1=pmask[:],
                                       op=ALU.mult)
               nc.vector.tensor_reduce(out=acc[:, 1:2], in_=scrp[:], op=ALU.add,
                                       axis=mybir.AxisListType.X)
               nc.vector.tensor_tensor(out=scrp[:], in0=pl_rm[:], in1=ptgt[:],
                                       op=ALU.mult)
               nc.vector.tensor_reduce(out=acc[:, 4:5], in_=scrp[:], op=ALU.add,
                                       axis=mybir.AxisListType.X)
               # p match: (pl > 0) == ptgt
               gt = cp.tile([128, RC], f32)
               nc.vector.tensor_scalar(out=gt[:], in0=pl_rm[:], scalar1=0.0,
                                       scalar2=None, op0=ALU.is_gt)
               nc.vector.tensor_tensor(out=gt[:], in0=gt[:], in1=ptgt[:],
                                       op=ALU.is_equal)
               nc.vector.tensor_tensor(out=scrp[:], in0=gt[:], in1=pmask[:],
                                       op=ALU.mult)
               nc.vector.tensor_reduce(out=acc[:, 3:4], in_=scrp[:], op=ALU.add,
                                       axis=mybir.AxisListType.X)

              # q-phase
              if DBG_Q:
               with tc.tile_pool(name="qhp", bufs=1, space="PSUM") as qhp, \
                    tc.tile_pool(name="qlps", bufs=3, space="PSUM") as qlps, \
                    tc.tile_pool(name="qwk", bufs=2) as qwk:
                   for cc in range(5):
                       # qp in 256-col halves (2 PSUM banks) frees room for
                       # ql bufs=3 so the per-rr logit matmuls run ahead of
                       # the reductions without stalling the PE
                       qpre = qwk.tile([128, 4, 512], f16, tag="qpre")
                       for hh in range(2):
                           hs = slice(cc * 512 + hh * 256,
                                      cc * 512 + (hh + 1) * 256)
                           qp = qhp.tile([128, 4, 256], f32, space="PSUM",
                                         tag="qp")
                           for m in range(4):
                               nc.tensor.matmul(
                                   qp[:, m, :],
                                   lhsT=twn[:, m, :],
                                   rhs=idt4[:],
                                   start=True, stop=False)
                               for k in range(4):
                                   nc.tensor.matmul(
                                       qp[:, m, :],
                                       lhsT=wwwh[:, k, bass.ts(m, 128)],
                                       rhs=hslab[:, k, hs],
                                       start=False, stop=(k == 3))
                           nc.scalar.activation(
                               qpre[:, :, hh * 256:(hh + 1) * 256], qp[:],
                               AF.Relu)
                       for rr in range(4):
                           rc = cc * 4 + rr
                           ql = qlps.tile([128, V], f32, space="PSUM", tag="ql")
                           for k in range(4):
                               nc.tensor.matmul(ql[:, 0:512],
                                                lhsT=qpre[:, k, bass.ts(rr, 128)],
                                                rhs=wo[:, k, 0:512],
                                                start=(k == 0), stop=(k == 3))
                           for k in range(4):
                               nc.tensor.matmul(ql[:, 512:V],
                                                lhsT=qpre[:, k, bass.ts(rr, 128)],
                                                rhs=wo[:, k, 512:V],
                                                start=(k == 0), stop=(k == 3))
                           ohp = qwk.tile([128, V], f32, tag="ohp")
                           nc.vector.scalar_tensor_tensor(
                               out=ohp[:], in0=iota_f[:],
                               scalar=qtg[:, rc:rc + 1], in1=ql[:],
                               op0=ALU.is_equal, op1=ALU.mult,
                               accum_out=tl_all[:, rc:rc + 1])
                           esc = qwk.tile([128, V], f16, tag="esc")
                           nc.scalar.activation(esc[:], ql[:], AF.Exp,
                                                accum_out=se_all[:, rc:rc + 1])
                           # argmax match via Sign(tl - ql) on the scalar
                           # engine: sum = #lt - #gt over V-1 non-target
                           # logits (target gives Sign(0)=0), so
                           # argmax==tgt  <=>  sum == V-1.
                           sg = qwk.tile([128, V], f16, tag="sg")
                           nc.scalar.activation(sg[:], ql[:], AF.Sign,
                                                bias=tl_all[:, rc:rc + 1],
                                                scale=-1.0,
                                                accum_out=cnt_all[:, rc:rc + 1])

              # ---- final reductions ----
              if DBG_FIN:
               fin = cp.tile([128, RC], f32)
               # lse = ln(se) ; qterm = (lse - tl)*qmask summed
               nc.scalar.activation(fin[:], se_all[:], AF.Ln)
               nc.vector.tensor_sub(fin[:], fin[:], tl_all[:])
               scr = cp.tile([128, RC], f32)
               nc.vector.tensor_tensor(out=scr[:], in0=fin[:], in1=qmask[:],
                                       op=ALU.mult)
               nc.vector.tensor_reduce(out=acc[:, 0:1], in_=scr[:], op=ALU.add,
                                       axis=mybir.AxisListType.X)
               # q match: sum of Sign(tl - ql) == V-1
               nc.vector.tensor_scalar(out=fin[:], in0=cnt_all[:],
                                       scalar1=float(V - 1),
                                       scalar2=None, op0=ALU.is_equal)
               nc.vector.tensor_tensor(out=scr[:], in0=fin[:], in1=qmask[:],
                                       op=ALU.mult)
               nc.vector.tensor_reduce(out=acc[:, 2:3], in_=scr[:], op=ALU.add,
                                       axis=mybir.AxisListType.X)

               with tc.tile_pool(name="fps", bufs=1, space="PSUM") as fps:
                   fin_ps = fps.tile([1, 8], f32, space="PSUM")
                   nc.tensor.matmul(fin_ps[:], lhsT=ones32[:], rhs=acc[:],
                                    start=True, stop=True)
                   fin_sb = cp.tile([1, 8], f32)
                   nc.vector.tensor_copy(fin_sb[:], fin_ps[:])
                   nc.sync.dma_start(out=out_d[:], in_=fin_sb[:])

    nc.compile()
    return nc


_NC_CACHE = {}
LAST_EXEC_NS = None
LAST_RES = None


def kernel(**inputs):
    cores, nch, gcnts = _host_prep(inputs)
    key = (tuple(nch), tuple(gcnts), DBG_T, DBG_HEADS, DBG_Q, DBG_FIN, GPELEM)
    if key not in _NC_CACHE:
        _NC_CACHE[key] = _build_program(tuple(nch), tuple(gcnts))
    nc = _NC_CACHE[key]
    in_maps = [{k: np.ascontiguousarray(v) for k, v in cores[c].items()}
               for c in range(NC)]
    trace = os.environ.get("KERNEL_TRACE", "0") == "1"
    res = run_bass_kernel_spmd(nc, in_maps, core_ids=list(range(NC)),
                               trace=trace)
    global LAST_EXEC_NS, LAST_RES
    LAST_EXEC_NS = getattr(res, "exec_time_ns", None)
    LAST_RES = res
    total = np.zeros(8, np.float64)
    for r in res.results:
        total += np.asarray(r["out"], np.float64).reshape(-1)
    q_loss = total[0] / B
    p_loss = (total[1] - total[4]) / B
    q_acc = total[2] / 10240.0
    p_acc = total[3] / (39 * B)
    return np.array([q_loss, p_loss, q_acc, p_acc], np.float32)


if __name__ == "__main__":
    pass
